# revision 35
# baseline (speedup 1.0000x reference)
# MoE routing + sparse-frequency inverse FFT2 kernel for Trainium2 (8 NeuronCores).
#
# Math: out_b = ALPHA * Re(ifft2(mask_b)) where mask_b has 4096 nonzero
# frequencies (top-2 experts x 2048 each).  With the symmetric real DFT basis
#   C[x,u] = cos(2*pi*x*u/768)/768,  S[x,u] = sin(2*pi*x*u/768)/768
# the dense iFFT2 factorizes into per-sample matmuls:
#   P = ALPHA*(M @ C), Qn = -ALPHA*(M @ S);  T1 = C @ P, T2 = S @ Qn
#   out[:, y]   = (T1+T2)[:, y]            for y in [0, 386)
#   out[:, N-y] = (T1-T2)[:, y]            (column symmetry: C even, S odd)
#   out[N-x, :] mirrors via shifted anti-identity matmuls on (T1-T2 | rev(T1+T2))
# All heavy matmuls run in fp16 (1 cycle/row on PE vs ~1.5+overhead for f32r;
# integer indices <= 2048 are exact in fp16 so iota/compare one-hots stay
# exact).  Stage-1/2 compute only 386 of 768 columns; the rest is add/sub +
# reversed-stride copies.
# Device work per core (4 samples): router GEMM, top-2 selection and weights,
# per-expert entry gather (one fused row-granular indirect DMA per slot),
# sparse->dense mask build via iota/compare one-hots placed with PE matmuls,
# then the fp16 matmul pipeline above.  Host only prepares input-layout
# constants: fp16 C/S tables (512 cols), a bucketed, padded, partition-major,
# u/vm/cv-interleaved re-layout of the static (list_indices, coeff) tables,
# plus batch sharding.

import sys

sys.path.insert(0, "/opt/trn_rl_repo")

import numpy as np

import concourse.bacc as bacc
import concourse.mybir as mybir
import concourse.tile as tile
from concourse.bass import IndirectOffsetOnAxis
from concourse.bass_utils import run_bass_kernel_spmd
from concourse.masks import make_identity

N = 768
E = 64
NF = 2048
B = 32
NCORES = 8
BPC = B // NCORES          # samples per core
NBLK = 6                   # 768 / 128
ALPHA = 300.0
GRID = N * N
HALF = N // 2 + 2          # 386 computed stage-1/2 columns (even width)
TCOL = 512                 # stored C/S table columns (stage-2 lhsT needs 512)

# per-(expert, v-chunk, u-chunk) buckets; expected fill 2048/36 ~ 57 (sigma
# ~7.3), padded to 128 so each (slot, bucket) build matmul is one 128-wide
# N=128 fp16 instruction that never crosses a PSUM bank boundary.
BPAD = 128                               # entries per bucket
BROW = NBLK * BPAD                       # 768 entries per (expert, v-chunk)
EROW = NBLK * BROW                       # 4608 entries per expert
COLS = EROW // 128                       # 36 gather columns per expert slot
GCOLS = 3 * COLS                         # umod | vm | cv interleaved per partition

F32 = mybir.dt.float32
F16 = mybir.dt.float16
I32 = mybir.dt.int32
AOT = mybir.AluOpType
REV = N - HALF             # 382 mirrored columns

KERNEL_TRACE = False       # test harness can flip this to profile
LAST_RESULT = None

_NC = None


def _build():
    nc = bacc.Bacc(trn_type="TRN2")

    cls4t = nc.dram_tensor("cls4t", [128, NBLK * BPC], F32, kind="ExternalInput")
    wrt = nc.dram_tensor("wrt", [128, NBLK * E], F32, kind="ExternalInput")
    br = nc.dram_tensor("br", [E], F32, kind="ExternalInput")
    ft = nc.dram_tensor("ft", [E, 3 * EROW], F32, kind="ExternalInput")
    bases = nc.dram_tensor("bases", [E, 1], F32, kind="ExternalInput")
    jm = nc.dram_tensor("jm", [128, 128], F16, kind="ExternalInput")
    ct = nc.dram_tensor("ct", [128, NBLK * TCOL], F16, kind="ExternalInput")
    st = nc.dram_tensor("st", [128, NBLK * TCOL], F16, kind="ExternalInput")
    out4 = nc.dram_tensor("out4", [BPC, N, N], F32, kind="ExternalOutput")

    with tile.TileContext(nc) as tc:
        with (
            tc.tile_pool(name="const", bufs=1) as cpool,
            tc.tile_pool(name="tables", bufs=1) as tpool,
            tc.tile_pool(name="routing", bufs=1) as rpool,
            tc.tile_pool(name="gath", bufs=1) as gpool,
            tc.tile_pool(name="build", bufs=2) as bpool,
            tc.tile_pool(name="mt", bufs=2) as mtpool,
            tc.tile_pool(name="pq", bufs=2) as pqpool,
            tc.tile_pool(name="outp", bufs=3) as opool,
            tc.tile_pool(name="psA", bufs=3, space="PSUM") as psA,
            tc.tile_pool(name="psA1", bufs=4, space="PSUM") as psA1,
            tc.tile_pool(name="psB", bufs=1, space="PSUM") as psB,
            tc.tile_pool(name="mir", bufs=4) as mirpool,
            tc.tile_pool(name="t1p", bufs=2) as t1pool,
        ):
            ident = cpool.tile([128, 128], F32)
            make_identity(nc, ident[:])
            ones1 = cpool.tile([1, 128], F32)
            nc.vector.memset(ones1[:], 1.0)
            ones14 = cpool.tile([1, BPC], F32)
            nc.vector.memset(ones14[:], 1.0)
            i128 = cpool.tile([128, 128], I32)
            nc.gpsimd.iota(i128[:], pattern=[[1, 128]], base=0, channel_multiplier=0)
            i128h = cpool.tile([128, 128], F16)
            nc.vector.tensor_copy(i128h[:], i128[:])
            io72 = cpool.tile([128, 1], I32)
            nc.gpsimd.iota(io72[:], pattern=[[0, 1]], base=0, channel_multiplier=GCOLS)
            io72f = cpool.tile([128, 1], F32)
            nc.vector.tensor_copy(io72f[:], io72[:])

            br_sb = rpool.tile([1, E], F32)
            nc.sync.dma_start(out=br_sb[:], in_=br[None, :])
            bases_sb = rpool.tile([E, 1], F32)
            nc.sync.dma_start(out=bases_sb[:], in_=bases[:])
            jJ = cpool.tile([128, 128], F16)
            nc.sync.dma_start(out=jJ[:], in_=jm[:])

            # ---- router: logits = (cls4T)^T @ WrT + br, both pre-transposed
            # on host so no device transposes sit on the critical path
            clst_sb = rpool.tile([128, NBLK * BPC], F32)
            wrt_sb = rpool.tile([128, NBLK * E], F32)
            nc.sync.dma_start(out=clst_sb[:], in_=cls4t[:])
            nc.sync.dma_start(out=wrt_sb[:], in_=wrt[:])
            lg_ps = psB.tile([BPC, E], F32, tag="small")
            for j in range(NBLK):
                nc.tensor.matmul(
                    lg_ps[:],
                    lhsT=clst_sb[:, BPC * j : BPC * (j + 1)],
                    rhs=wrt_sb[:, E * j : E * (j + 1)],
                    start=(j == 0),
                    stop=False,
                )
            nc.tensor.matmul(
                lg_ps[:], lhsT=ones14[:], rhs=br_sb[:], start=False, stop=True
            )
            logits = rpool.tile([BPC, E], F32)
            nc.vector.tensor_copy(logits[:], lg_ps[:])

            # ---- top-2, renormalized weights, one-hot selectors ----
            max8 = rpool.tile([BPC, 8], F32)
            nc.vector.max(out=max8[:], in_=logits[:])
            l0 = max8[:, 0:1]
            l1 = max8[:, 1:2]
            d = rpool.tile([BPC, 1], F32)
            nc.vector.tensor_sub(d[:], l1, l0)  # l1 - l0
            dT_ps = psB.tile([1, BPC], F32, tag="small")
            nc.tensor.transpose(dT_ps[:], d[:], ident[0:BPC, 0:BPC])
            dT = rpool.tile([1, BPC], F32)
            nc.vector.tensor_copy(dT[:], dT_ps[:])
            w1T = rpool.tile([1, BPC], F32)
            nc.scalar.activation(w1T[:], dT[:], mybir.ActivationFunctionType.Sigmoid)
            w0T = rpool.tile([1, BPC], F32)
            nc.scalar.activation(
                w0T[:], dT[:], mybir.ActivationFunctionType.Sigmoid, scale=-1.0
            )
            oh1 = rpool.tile([BPC, E], F32)
            oh2 = rpool.tile([BPC, E], F32)
            nc.vector.tensor_scalar(oh1[:], logits[:], l0, None, op0=AOT.is_equal)
            nc.vector.tensor_scalar(oh2[:], logits[:], l1, None, op0=AOT.is_equal)
            selT = []
            for srcap in (oh1, oh2):
                sp = psB.tile([E, BPC], F32, tag="small")
                nc.tensor.transpose(sp[:], srcap[:], ident[0:BPC, 0:BPC])
                sbt = rpool.tile([E, BPC], F32, tag=f"selT{len(selT)}")
                nc.vector.tensor_copy(sbt[:], sp[:])
                selT.append(sbt)
            o1T, o2T = selT

            # per-sample scalar rows [1, BPC]: expert table offsets
            eT = []
            for oT in (o1T, o2T):
                ep = psB.tile([1, BPC], F32, tag="small")
                nc.tensor.matmul(
                    ep[:], lhsT=bases_sb[:], rhs=oT[:], start=True, stop=True
                )
                es = rpool.tile([1, BPC], F32, tag=f"eT{len(eT)}")
                nc.vector.tensor_copy(es[:], ep[:])
                eT.append(es)

            # broadcast all 4 scalar rows to 128 partitions in one matmul
            rows4 = rpool.tile([1, 4 * BPC], F32)
            for ri, rowap in enumerate((eT[0], eT[1], w0T, w1T)):
                nc.vector.tensor_copy(rows4[:, BPC * ri : BPC * (ri + 1)], rowap[:])
            bp = psB.tile([128, 4 * BPC], F32, tag="small")
            nc.tensor.matmul(bp[:], lhsT=ones1[:], rhs=rows4[:], start=True, stop=True)
            bcast = rpool.tile([128, 4 * BPC], F32)
            nc.vector.tensor_copy(bcast[:], bp[:])
            ebc = [bcast[:, 0:BPC], bcast[:, BPC : 2 * BPC]]
            wbc = [bcast[:, 2 * BPC : 3 * BPC], bcast[:, 3 * BPC : 4 * BPC]]

            # ---- C/S table loads AFTER the routing-phase emission so the
            # small router DMAs aren't queued behind bulk on the sync FIFO
            ct_sb = tpool.tile([128, NBLK * TCOL], F16, tag="ct")
            st_sb = tpool.tile([128, NBLK * TCOL], F16, tag="st")
            nc.sync.dma_start(out=ct_sb[:], in_=ct[:])
            nc.sync.dma_start(out=st_sb[:], in_=st[:])

            def emit_d(b, di, mc):
                ob = opool.tile([128, N], F32, tag="ob")
                for c0 in (0, 384):
                    dps = psA1.tile([128, HALF], F32, tag="mm1")
                    nc.tensor.matmul(
                        dps[:, 0:384],
                        lhsT=jJ[:],
                        rhs=mc[:, c0 : c0 + 384],
                        start=True, stop=True,
                    )
                    nc.scalar.copy(ob[:, c0 : c0 + 384], dps[:, 0:384])
                nc.sync.dma_start(
                    out=out4[:][b][128 * (4 + di) : 128 * (5 + di), :], in_=ob[:]
                )

            # ---- gather ALL samples' (u, vm, coeff) entry tables upfront ----
            # one fused indirect DMA per (sample, slot): per-partition run of
            # COLS u-values, COLS v-mod values, COLS coefficients
            offf8 = gpool.tile([128, 2 * BPC], F32, tag="offf")
            nc.vector.tensor_scalar(
                offf8[:], bcast[:, 0 : 2 * BPC], io72f[:], None, op0=AOT.add
            )
            offs8 = gpool.tile([128, 2 * BPC], I32, tag="offs")
            nc.vector.tensor_copy(offs8[:], offf8[:])
            # per sample: one fused [u|vm|cv] gather per slot into one tile,
            # then THREE wide broadcast-AP compares build every bucket's
            # one-hot pair at once (vs 144 tiny per-bucket DVE ops)
            allg = []
            for b in range(BPC):
                gg2 = gpool.tile([128, 2 * GCOLS], F32, tag=f"gg{b}")
                for slot in range(2):
                    nc.gpsimd.indirect_dma_start(
                        out=gg2[:, GCOLS * slot : GCOLS * (slot + 1)],
                        out_offset=None,
                        in_=ft[:],
                        in_offset=IndirectOffsetOnAxis(
                            ap=offs8[:, BPC * slot + b : BPC * slot + b + 1], axis=1
                        ),
                    )
                gcw2 = gpool.tile([128, 2 * COLS], F32, tag=f"gcw{b}")
                for slot in range(2):
                    nc.vector.tensor_scalar(
                        gcw2[:, COLS * slot : COLS * (slot + 1)],
                        gg2[:, GCOLS * slot + 2 * COLS : GCOLS * slot + 3 * COLS],
                        wbc[slot][:, b : b + 1], None, op0=AOT.mult,
                    )
                allg.append((gg2, gcw2))

            i4 = (
                i128h[:].unsqueeze(1).unsqueeze(1).broadcast_to([128, 2, COLS, 128])
            )

            pending = None
            for b in range(BPC):
                gg2, gcw2 = allg[b]
                # wide one-hot builds: all 72 (slot, bucket) pairs in 3 DVE ops
                vall = bpool.tile([128, 2 * COLS * 128], F16, tag="vall")
                uall = bpool.tile([128, 2 * COLS * 128], F16, tag="uall")
                g4 = gg2[:].rearrange("p (s t c) -> p s t c", s=2, t=3)
                v4 = vall[:].rearrange("p (s k v) -> p s k v", s=2, k=COLS)
                u4 = uall[:].rearrange("p (s k v) -> p s k v", s=2, k=COLS)
                nc.vector.tensor_tensor(
                    v4,
                    i4,
                    g4[:, :, 1, :].unsqueeze(3).broadcast_to([128, 2, COLS, 128]),
                    op=AOT.is_equal,
                )
                nc.vector.tensor_tensor(
                    u4,
                    i4,
                    g4[:, :, 0, :].unsqueeze(3).broadcast_to([128, 2, COLS, 128]),
                    op=AOT.is_equal,
                )
                nc.vector.tensor_tensor(
                    u4,
                    u4,
                    gcw2[:].rearrange("p (s c) -> p s c", s=2)
                    .unsqueeze(3).broadcast_to([128, 2, COLS, 128]),
                    op=AOT.mult,
                )

                # ---- build MT (transposed mask) chunk by chunk on PE ----
                mt_sb = mtpool.tile([128, NBLK * N], F16, tag="mt")
                for j in range(NBLK):
                    for half in range(2):
                        mtps = psA.tile([128, 384], F32, tag="mm")
                        for ui in range(3):
                            ub = 3 * half + ui
                            for slot in range(2):
                                base = 4608 * slot + 128 * (NBLK * j + ub)
                                nc.tensor.matmul(
                                    mtps[:, 128 * ui : 128 * (ui + 1)],
                                    lhsT=vall[:, base : base + 128],
                                    rhs=uall[:, base : base + 128],
                                    start=(slot == 0),
                                    stop=(slot == 1),
                                )
                        nc.scalar.copy(
                            mt_sb[:, N * j + 384 * half : N * j + 384 * (half + 1)],
                            mtps[:],
                        )

                # emit the PREVIOUS sample's mirrored rows here so those PE
                # instructions sit behind the (vector-gated) mask build and
                # never stall on the mirror-patch chain
                if pending is not None:
                    pb, pmirs = pending
                    emit_d(pb, 0, pmirs[1])
                    emit_d(pb, 1, pmirs[0])
                    pending = None

                # ---- stage 1: P = 300*(M @ C), Qn = -300*(M @ S), cols [0,386)
                p_sb = pqpool.tile([128, NBLK * HALF], F16, tag="p")
                q_sb = pqpool.tile([128, NBLK * HALF], F16, tag="q")
                for i in range(NBLK):
                    pps = psA1.tile([128, HALF], F32, tag="mm1")
                    qps = psA1.tile([128, HALF], F32, tag="mm1")
                    for k in range(NBLK):
                        lhs = mt_sb[:, N * k + 128 * i : N * k + 128 * (i + 1)]
                        nc.tensor.matmul(
                            pps[:], lhsT=lhs, rhs=ct_sb[:, TCOL * k : TCOL * k + HALF],
                            start=(k == 0), stop=(k == NBLK - 1),
                        )
                        nc.tensor.matmul(
                            qps[:], lhsT=lhs, rhs=st_sb[:, TCOL * k : TCOL * k + HALF],
                            start=(k == 0), stop=(k == NBLK - 1),
                        )
                    nc.scalar.mul(p_sb[:, HALF * i : HALF * (i + 1)], pps[:], ALPHA)
                    nc.scalar.mul(q_sb[:, HALF * i : HALF * (i + 1)], qps[:], -ALPHA)

                # ---- stage 2: rows 0..511 via T1/T2 on 386 cols; columns
                # 386..767 by symmetry; rows 512..767 mirrored via emit_d.
                mirs = []
                for i in range(4):
                    t1 = psA1.tile([128, HALF], F32, tag="mm1")
                    t2 = psA1.tile([128, HALF], F32, tag="mm1")
                    for dst, tbl, srcm in ((t1, ct_sb, p_sb), (t2, st_sb, q_sb)):
                        for k in range(NBLK):
                            nc.tensor.matmul(
                                dst[:],
                                lhsT=tbl[:, TCOL * k + 128 * i : TCOL * k + 128 * (i + 1)],
                                rhs=srcm[:, HALF * k : HALF * (k + 1)],
                                start=(k == 0),
                                stop=(k == NBLK - 1),
                            )
                    # PSUM is released via the fast scalar queue (t1s/t2s);
                    # every later reconstruction op reads SBUF only, so the
                    # vector queue can lag without stalling the PSUM ring.
                    t1s = t1pool.tile([128, HALF], F16, tag="t1")
                    nc.scalar.copy(t1s[:], t1[:])
                    t2s = t1pool.tile([128, HALF], F16, tag="t2")
                    nc.scalar.copy(t2s[:], t2[:])
                    ob = opool.tile([128, N], F32, tag="ob")
                    nc.vector.tensor_tensor(ob[:, 0:HALF], t1s[:], t2s[:], op=AOT.add)
                    # out[:, y] = (T1-T2)[:, N-y] for y in [386, 768): reversed dst
                    nc.vector.tensor_tensor(
                        ob[:][:, N - 1 : HALF - 1 : -1],
                        t1s[:, 1 : REV + 1], t2s[:, 1 : REV + 1], op=AOT.subtract,
                    )
                    if i < 2:
                        m = mirpool.tile([128, N], F16, tag=f"mc{i}")
                        nc.vector.tensor_tensor(
                            m[:, 0:HALF], t1s[:], t2s[:], op=AOT.subtract
                        )
                        nc.vector.tensor_tensor(
                            m[:][:, N - 1 : HALF - 1 : -1],
                            t1s[:, 1 : REV + 1], t2s[:, 1 : REV + 1], op=AOT.add,
                        )
                        mirs.append(m)
                    elif i == 2:
                        # mc1 row 0 = block-2 row 0 (x = 256)
                        nc.vector.tensor_tensor(
                            mirs[1][0:1, 0:HALF], t1s[0:1, :], t2s[0:1, :],
                            op=AOT.subtract,
                        )
                        nc.vector.tensor_tensor(
                            mirs[1][0:1, :][:, N - 1 : HALF - 1 : -1],
                            t1s[0:1, 1 : REV + 1], t2s[0:1, 1 : REV + 1], op=AOT.add,
                        )
                    nc.sync.dma_start(
                        out=out4[:][b][128 * i : 128 * (i + 1), :], in_=ob[:]
                    )
                    if i == 1:
                        # mc0 row 0 = block-1 row 0 (x = 128)
                        nc.scalar.copy(mirs[0][0:1, :], m[0:1, :])
                pending = (b, mirs)
            pb, pmirs = pending
            emit_d(pb, 0, pmirs[1])
            emit_d(pb, 1, pmirs[0])

    nc.compile()
    return nc


def _get_nc():
    global _NC
    if _NC is None:
        _NC = _build()
    return _NC


def _pack_pm(a):
    """[768, W] row-major -> [128, 6*W] with 128-row block j at cols j*W."""
    w = a.shape[1]
    return np.ascontiguousarray(
        a.reshape(NBLK, 128, w).transpose(1, 0, 2).reshape(128, NBLK * w)
    )


def _host_tables():
    a = np.arange(N, dtype=np.int64)
    ang = (2.0 * np.pi / N) * ((a[:, None] * a[None, :]) % N)
    ctv = (np.cos(ang) / N).astype(np.float16)[:, 0:TCOL]
    stv = (np.sin(ang) / N).astype(np.float16)[:, 0:TCOL]
    return _pack_pm(ctv), _pack_pm(stv)


def _host_entry_tables(list_indices, coeff):
    """Bucket each expert's (u, v, coeff) entries by (v-chunk, u-half), pad
    buckets to PAD, lay out partition-major, and interleave u/vm/cv per
    partition so one indirect DMA fetches all three."""
    li = list_indices.astype(np.int64)
    uu = li // N
    vv = li % N
    u2 = np.full((E, EROW), -9.0, np.float32)
    vm2 = np.full((E, EROW), -9.0, np.float32)
    cv2 = np.zeros((E, EROW), np.float32)
    for e in range(E):
        for j in range(NBLK):
            selj = vv[e] // 128 == j
            for ub in range(NBLK):
                sel = np.where(selj & (uu[e] // 128 == ub))[0]
                cnt = len(sel)
                assert cnt <= BPAD, f"bucket overflow: e{e} j{j} ub{ub}: {cnt}"
                base = BROW * j + BPAD * ub
                u2[e, base : base + cnt] = uu[e, sel] - 128 * ub
                vm2[e, base : base + cnt] = vv[e, sel] - 128 * j
                cv2[e, base : base + cnt] = coeff[e, sel]
    # fused layout: ftab[e, p*GCOLS + t*COLS + g] = arr_t[e, 128*g + p]
    ftab = np.zeros((E, 3 * EROW), np.float32)
    p_ix = np.arange(128)[:, None]
    g_ix = np.arange(COLS)[None, :]
    src = (128 * g_ix + p_ix).reshape(-1)          # [128*COLS] entry index
    for t, arr in enumerate((u2, vm2, cv2)):
        dst = (p_ix * GCOLS + t * COLS + g_ix).reshape(-1)
        ftab[:, dst] = arr[:, src]
    return ftab


def kernel(cls_token, W_router, b_router, coeff, list_indices):
    global LAST_RESULT
    cls_token = np.asarray(cls_token)
    W_router = np.asarray(W_router)
    b_router = np.asarray(b_router)
    coeff = np.asarray(coeff)
    list_indices = np.asarray(list_indices)
    assert cls_token.shape == (B, N) and coeff.shape == (E, NF)
    nc = _get_nc()
    ctv, stv = _host_tables()
    ftv = _host_entry_tables(list_indices, coeff)
    basesv = (np.arange(E, dtype=np.float32) * (3 * EROW)).reshape(E, 1)
    jmv = np.zeros((128, 128), np.float16)
    for m_ in range(128):
        jmv[(128 - m_) % 128, m_] = 1.0
    wrtv = _pack_pm(W_router.T.astype(np.float32))
    brr = np.ascontiguousarray(b_router, dtype=np.float32)
    in_maps = []
    for c in range(NCORES):
        in_maps.append(
            {
                "cls4t": _pack_pm(
                    cls_token[BPC * c : BPC * (c + 1)].T.astype(np.float32)
                ),
                "wrt": wrtv,
                "br": brr,
                "ft": ftv,
                "bases": basesv,
                "jm": jmv,
                "ct": ctv,
                "st": stv,
            }
        )
    res = run_bass_kernel_spmd(
        nc, in_maps, core_ids=list(range(NCORES)), trace=KERNEL_TRACE
    )
    LAST_RESULT = res
    out = np.concatenate([res.results[c]["out4"] for c in range(NCORES)], axis=0)
    return out


# revision 38
# speedup vs baseline: 1.1266x; 1.1266x over previous
# MoE routing + sparse-frequency inverse FFT2 kernel for Trainium2 (8 NeuronCores).
#
# Math: out_b = ALPHA * Re(ifft2(mask_b)) where mask_b has 4096 nonzero
# frequencies (top-2 experts x 2048 each).  With the symmetric real DFT basis
#   C[x,u] = cos(2*pi*x*u/768)/768,  S[x,u] = sin(2*pi*x*u/768)/768
# the dense iFFT2 factorizes into per-sample matmuls:
#   P = ALPHA*(M @ C), Qn = -ALPHA*(M @ S);  T1 = C @ P, T2 = S @ Qn
#   out[:, y]   = (T1+T2)[:, y]            for y in [0, 386)
#   out[:, N-y] = (T1-T2)[:, y]            (column symmetry: C even, S odd)
#   out[N-x, :] mirrors via shifted anti-identity matmuls on (T1-T2 | rev(T1+T2))
# All heavy matmuls run in fp16 (1 cycle/row on PE vs ~1.5+overhead for f32r;
# integer indices <= 2048 are exact in fp16 so iota/compare one-hots stay
# exact).  Stage-1/2 compute only 386 of 768 columns; the rest is add/sub +
# reversed-stride copies.
# Device work per core (4 samples): router GEMM, top-2 selection and weights,
# per-expert entry gather (one fused row-granular indirect DMA per slot),
# sparse->dense mask build via iota/compare one-hots placed with PE matmuls,
# then the fp16 matmul pipeline above.  Host only prepares input-layout
# constants: fp16 C/S tables (512 cols), a bucketed, padded, partition-major,
# u/vm/cv-interleaved re-layout of the static (list_indices, coeff) tables,
# plus batch sharding.

import sys

sys.path.insert(0, "/opt/trn_rl_repo")

import numpy as np

import concourse.bacc as bacc
import concourse.mybir as mybir
import concourse.tile as tile
from concourse.bass import IndirectOffsetOnAxis
from concourse.bass_utils import run_bass_kernel_spmd
from concourse.masks import make_identity

N = 768
E = 64
NF = 2048
B = 32
NCORES = 8
BPC = B // NCORES          # samples per core
NBLK = 6                   # 768 / 128
ALPHA = 300.0
GRID = N * N
HALF = N // 2 + 2          # 386 computed stage-1/2 columns (even width)
TCOL = 512                 # stored C/S table columns (stage-2 lhsT needs 512)

# per-(expert, v-chunk, u-chunk) buckets; expected fill 2048/36 ~ 57 (sigma
# ~7.3), padded to 128 so each (slot, bucket) build matmul is one 128-wide
# N=128 fp16 instruction that never crosses a PSUM bank boundary.
BPAD = 128                               # entries per bucket
BROW = NBLK * BPAD                       # 768 entries per (expert, v-chunk)
EROW = NBLK * BROW                       # 4608 entries per expert
COLS = EROW // 128                       # 36 gather columns per expert slot
GCOLS = 3 * COLS                         # umod | vm | cv interleaved per partition

F32 = mybir.dt.float32
F16 = mybir.dt.float16
I32 = mybir.dt.int32
AOT = mybir.AluOpType
REV = N - HALF             # 382 mirrored columns

KERNEL_TRACE = False       # test harness can flip this to profile
LAST_RESULT = None

_NC = None


def _build():
    nc = bacc.Bacc(trn_type="TRN2")

    cls4t = nc.dram_tensor("cls4t", [128, NBLK * BPC], F32, kind="ExternalInput")
    wrt = nc.dram_tensor("wrt", [128, NBLK * E], F32, kind="ExternalInput")
    br = nc.dram_tensor("br", [E], F32, kind="ExternalInput")
    ft = nc.dram_tensor("ft", [E, 3 * EROW], F32, kind="ExternalInput")
    bases = nc.dram_tensor("bases", [E, 1], F32, kind="ExternalInput")
    jm = nc.dram_tensor("jm", [128, 128], F16, kind="ExternalInput")
    ct = nc.dram_tensor("ct", [128, NBLK * TCOL], F16, kind="ExternalInput")
    st = nc.dram_tensor("st", [128, NBLK * TCOL], F16, kind="ExternalInput")
    out4 = nc.dram_tensor("out4", [BPC, N, N], F32, kind="ExternalOutput")

    with tile.TileContext(nc) as tc:
        with (
            tc.tile_pool(name="const", bufs=1) as cpool,
            tc.tile_pool(name="tables", bufs=1) as tpool,
            tc.tile_pool(name="routing", bufs=1) as rpool,
            tc.tile_pool(name="gath", bufs=1) as gpool,
            tc.tile_pool(name="build", bufs=48) as bpool,
            tc.tile_pool(name="mt", bufs=2) as mtpool,
            tc.tile_pool(name="pq", bufs=2) as pqpool,
            tc.tile_pool(name="outp", bufs=3) as opool,
            tc.tile_pool(name="psA", bufs=3, space="PSUM") as psA,
            tc.tile_pool(name="psA1", bufs=4, space="PSUM") as psA1,
            tc.tile_pool(name="psB", bufs=1, space="PSUM") as psB,
            tc.tile_pool(name="mir", bufs=4) as mirpool,
            tc.tile_pool(name="t1p", bufs=2) as t1pool,
        ):
            ident = cpool.tile([128, 128], F32)
            make_identity(nc, ident[:])
            ones1 = cpool.tile([1, 128], F32)
            nc.vector.memset(ones1[:], 1.0)
            ones14 = cpool.tile([1, BPC], F32)
            nc.vector.memset(ones14[:], 1.0)
            i128 = cpool.tile([128, 128], I32)
            nc.gpsimd.iota(i128[:], pattern=[[1, 128]], base=0, channel_multiplier=0)
            i128h = cpool.tile([128, 128], F16)
            nc.vector.tensor_copy(i128h[:], i128[:])
            io72 = cpool.tile([128, 1], I32)
            nc.gpsimd.iota(io72[:], pattern=[[0, 1]], base=0, channel_multiplier=GCOLS)
            io72f = cpool.tile([128, 1], F32)
            nc.vector.tensor_copy(io72f[:], io72[:])

            br_sb = rpool.tile([1, E], F32)
            nc.sync.dma_start(out=br_sb[:], in_=br[None, :])
            bases_sb = rpool.tile([E, 1], F32)
            nc.sync.dma_start(out=bases_sb[:], in_=bases[:])
            jJ = cpool.tile([128, 128], F16)
            nc.sync.dma_start(out=jJ[:], in_=jm[:])

            # ---- router: logits = (cls4T)^T @ WrT + br, both pre-transposed
            # on host so no device transposes sit on the critical path
            clst_sb = rpool.tile([128, NBLK * BPC], F32)
            wrt_sb = rpool.tile([128, NBLK * E], F32)
            nc.sync.dma_start(out=clst_sb[:], in_=cls4t[:])
            nc.sync.dma_start(out=wrt_sb[:], in_=wrt[:])
            lg_ps = psB.tile([BPC, E], F32, tag="small")
            for j in range(NBLK):
                nc.tensor.matmul(
                    lg_ps[:],
                    lhsT=clst_sb[:, BPC * j : BPC * (j + 1)],
                    rhs=wrt_sb[:, E * j : E * (j + 1)],
                    start=(j == 0),
                    stop=False,
                )
            nc.tensor.matmul(
                lg_ps[:], lhsT=ones14[:], rhs=br_sb[:], start=False, stop=True
            )
            logits = rpool.tile([BPC, E], F32)
            nc.vector.tensor_copy(logits[:], lg_ps[:])

            # ---- top-2, renormalized weights, one-hot selectors ----
            max8 = rpool.tile([BPC, 8], F32)
            nc.vector.max(out=max8[:], in_=logits[:])
            l0 = max8[:, 0:1]
            l1 = max8[:, 1:2]
            d = rpool.tile([BPC, 1], F32)
            nc.vector.tensor_sub(d[:], l1, l0)  # l1 - l0
            dT_ps = psB.tile([1, BPC], F32, tag="small")
            nc.tensor.transpose(dT_ps[:], d[:], ident[0:BPC, 0:BPC])
            dT = rpool.tile([1, BPC], F32)
            nc.vector.tensor_copy(dT[:], dT_ps[:])
            w1T = rpool.tile([1, BPC], F32)
            nc.scalar.activation(w1T[:], dT[:], mybir.ActivationFunctionType.Sigmoid)
            w0T = rpool.tile([1, BPC], F32)
            nc.scalar.activation(
                w0T[:], dT[:], mybir.ActivationFunctionType.Sigmoid, scale=-1.0
            )
            oh1 = rpool.tile([BPC, E], F32)
            oh2 = rpool.tile([BPC, E], F32)
            nc.vector.tensor_scalar(oh1[:], logits[:], l0, None, op0=AOT.is_equal)
            nc.vector.tensor_scalar(oh2[:], logits[:], l1, None, op0=AOT.is_equal)
            selT = []
            for srcap in (oh1, oh2):
                sp = psB.tile([E, BPC], F32, tag="small")
                nc.tensor.transpose(sp[:], srcap[:], ident[0:BPC, 0:BPC])
                sbt = rpool.tile([E, BPC], F32, tag=f"selT{len(selT)}")
                nc.vector.tensor_copy(sbt[:], sp[:])
                selT.append(sbt)
            o1T, o2T = selT

            # per-sample scalar rows [1, BPC]: expert table offsets
            eT = []
            for oT in (o1T, o2T):
                ep = psB.tile([1, BPC], F32, tag="small")
                nc.tensor.matmul(
                    ep[:], lhsT=bases_sb[:], rhs=oT[:], start=True, stop=True
                )
                es = rpool.tile([1, BPC], F32, tag=f"eT{len(eT)}")
                nc.vector.tensor_copy(es[:], ep[:])
                eT.append(es)

            # broadcast all 4 scalar rows to 128 partitions in one matmul
            rows4 = rpool.tile([1, 4 * BPC], F32)
            for ri, rowap in enumerate((eT[0], eT[1], w0T, w1T)):
                nc.vector.tensor_copy(rows4[:, BPC * ri : BPC * (ri + 1)], rowap[:])
            bp = psB.tile([128, 4 * BPC], F32, tag="small")
            nc.tensor.matmul(bp[:], lhsT=ones1[:], rhs=rows4[:], start=True, stop=True)
            bcast = rpool.tile([128, 4 * BPC], F32)
            nc.vector.tensor_copy(bcast[:], bp[:])
            ebc = [bcast[:, 0:BPC], bcast[:, BPC : 2 * BPC]]
            wbc = [bcast[:, 2 * BPC : 3 * BPC], bcast[:, 3 * BPC : 4 * BPC]]

            # ---- C/S table loads AFTER the routing-phase emission so the
            # small router DMAs aren't queued behind bulk on the sync FIFO
            ct_sb = tpool.tile([128, NBLK * TCOL], F16, tag="ct")
            st_sb = tpool.tile([128, NBLK * TCOL], F16, tag="st")
            nc.sync.dma_start(out=ct_sb[:], in_=ct[:])
            nc.sync.dma_start(out=st_sb[:], in_=st[:])

            def emit_d(b, di, mc):
                ob = opool.tile([128, N], F32, tag="ob")
                for c0 in (0, 384):
                    dps = psA1.tile([128, HALF], F32, tag="mm1")
                    nc.tensor.matmul(
                        dps[:, 0:384],
                        lhsT=jJ[:],
                        rhs=mc[:, c0 : c0 + 384],
                        start=True, stop=True,
                    )
                    nc.scalar.copy(ob[:, c0 : c0 + 384], dps[:, 0:384])
                nc.sync.dma_start(
                    out=out4[:][b][128 * (4 + di) : 128 * (5 + di), :], in_=ob[:]
                )

            # ---- gather ALL samples' (u, vm, coeff) entry tables upfront ----
            # one fused indirect DMA per (sample, slot): per-partition run of
            # COLS u-values, COLS v-mod values, COLS coefficients
            offf8 = gpool.tile([128, 2 * BPC], F32, tag="offf")
            nc.vector.tensor_scalar(
                offf8[:], bcast[:, 0 : 2 * BPC], io72f[:], None, op0=AOT.add
            )
            offs8 = gpool.tile([128, 2 * BPC], I32, tag="offs")
            nc.vector.tensor_copy(offs8[:], offf8[:])
            # per sample: one fused [u|vm|cv] gather per slot into one tile,
            # then THREE wide broadcast-AP compares build every bucket's
            # one-hot pair at once (vs 144 tiny per-bucket DVE ops)
            allg = []
            for b in range(BPC):
                gg2 = gpool.tile([128, 2 * GCOLS], F32, tag=f"gg{b}")
                for slot in range(2):
                    nc.gpsimd.indirect_dma_start(
                        out=gg2[:, GCOLS * slot : GCOLS * (slot + 1)],
                        out_offset=None,
                        in_=ft[:],
                        in_offset=IndirectOffsetOnAxis(
                            ap=offs8[:, BPC * slot + b : BPC * slot + b + 1], axis=1
                        ),
                    )
                gcw2 = gpool.tile([128, 2 * COLS], F32, tag=f"gcw{b}")
                for slot in range(2):
                    nc.vector.tensor_scalar(
                        gcw2[:, COLS * slot : COLS * (slot + 1)],
                        gg2[:, GCOLS * slot + 2 * COLS : GCOLS * slot + 3 * COLS],
                        wbc[slot][:, b : b + 1], None, op0=AOT.mult,
                    )
                allg.append((gg2, gcw2))

            pending = None
            for b in range(BPC):
                gg2, gcw2 = allg[b]
                # ---- build MT (transposed mask) chunk by chunk on PE ----
                mt_sb = mtpool.tile([128, NBLK * N], F16, tag="mt")
                for j in range(NBLK):
                    for half in range(2):
                        mtps = psA.tile([128, 384], F32, tag="mm")
                        for ui in range(3):
                            ub = 3 * half + ui
                            for slot in range(2):
                                col = NBLK * j + ub
                                voh = bpool.tile([128, 128], F16, tag="voh")
                                nc.vector.tensor_scalar(
                                    voh[:], i128h[:],
                                    gg2[:, GCOLS * slot + COLS + col
                                        : GCOLS * slot + COLS + col + 1],
                                    None, op0=AOT.is_equal,
                                )
                                rhsb = bpool.tile([128, 128], F16, tag="rhsb")
                                nc.vector.tensor_scalar(
                                    rhsb[:], i128h[:],
                                    gg2[:, GCOLS * slot + col
                                        : GCOLS * slot + col + 1],
                                    gcw2[:, COLS * slot + col
                                         : COLS * slot + col + 1],
                                    op0=AOT.is_equal, op1=AOT.mult,
                                )
                                nc.tensor.matmul(
                                    mtps[:, 128 * ui : 128 * (ui + 1)],
                                    lhsT=voh[:],
                                    rhs=rhsb[:],
                                    start=(slot == 0),
                                    stop=(slot == 1),
                                )
                        nc.scalar.copy(
                            mt_sb[:, N * j + 384 * half : N * j + 384 * (half + 1)],
                            mtps[:],
                        )

                # emit the PREVIOUS sample's mirrored rows here so those PE
                # instructions sit behind the (vector-gated) mask build and
                # never stall on the mirror-patch chain
                if pending is not None:
                    pb, pmirs = pending
                    emit_d(pb, 0, pmirs[1])
                    emit_d(pb, 1, pmirs[0])
                    pending = None

                # ---- stage 1: P = 300*(M @ C), Qn = -300*(M @ S), cols [0,386)
                p_sb = pqpool.tile([128, NBLK * HALF], F16, tag="p")
                q_sb = pqpool.tile([128, NBLK * HALF], F16, tag="q")
                for i in range(NBLK):
                    pps = psA1.tile([128, HALF], F32, tag="mm1")
                    qps = psA1.tile([128, HALF], F32, tag="mm1")
                    for k in range(NBLK):
                        lhs = mt_sb[:, N * k + 128 * i : N * k + 128 * (i + 1)]
                        nc.tensor.matmul(
                            pps[:], lhsT=lhs, rhs=ct_sb[:, TCOL * k : TCOL * k + HALF],
                            start=(k == 0), stop=(k == NBLK - 1),
                        )
                        nc.tensor.matmul(
                            qps[:], lhsT=lhs, rhs=st_sb[:, TCOL * k : TCOL * k + HALF],
                            start=(k == 0), stop=(k == NBLK - 1),
                        )
                    nc.scalar.mul(p_sb[:, HALF * i : HALF * (i + 1)], pps[:], ALPHA)
                    nc.scalar.mul(q_sb[:, HALF * i : HALF * (i + 1)], qps[:], -ALPHA)

                # ---- stage 2: rows 0..511 via T1/T2 on 386 cols; columns
                # 386..767 by symmetry; rows 512..767 mirrored via emit_d.
                mirs = []
                for i in range(4):
                    t1 = psA1.tile([128, HALF], F32, tag="mm1")
                    t2 = psA1.tile([128, HALF], F32, tag="mm1")
                    for dst, tbl, srcm in ((t1, ct_sb, p_sb), (t2, st_sb, q_sb)):
                        for k in range(NBLK):
                            nc.tensor.matmul(
                                dst[:],
                                lhsT=tbl[:, TCOL * k + 128 * i : TCOL * k + 128 * (i + 1)],
                                rhs=srcm[:, HALF * k : HALF * (k + 1)],
                                start=(k == 0),
                                stop=(k == NBLK - 1),
                            )
                    # PSUM is released via the fast scalar queue (t1s/t2s);
                    # every later reconstruction op reads SBUF only, so the
                    # vector queue can lag without stalling the PSUM ring.
                    t1s = t1pool.tile([128, HALF], F16, tag="t1")
                    nc.scalar.copy(t1s[:], t1[:])
                    t2s = t1pool.tile([128, HALF], F16, tag="t2")
                    nc.scalar.copy(t2s[:], t2[:])
                    ob = opool.tile([128, N], F32, tag="ob")
                    nc.vector.tensor_tensor(ob[:, 0:HALF], t1s[:], t2s[:], op=AOT.add)
                    # out[:, y] = (T1-T2)[:, N-y] for y in [386, 768): reversed dst
                    nc.vector.tensor_tensor(
                        ob[:][:, N - 1 : HALF - 1 : -1],
                        t1s[:, 1 : REV + 1], t2s[:, 1 : REV + 1], op=AOT.subtract,
                    )
                    if i < 2:
                        m = mirpool.tile([128, N], F16, tag=f"mc{i}")
                        nc.vector.tensor_tensor(
                            m[:, 0:HALF], t1s[:], t2s[:], op=AOT.subtract
                        )
                        nc.vector.tensor_tensor(
                            m[:][:, N - 1 : HALF - 1 : -1],
                            t1s[:, 1 : REV + 1], t2s[:, 1 : REV + 1], op=AOT.add,
                        )
                        mirs.append(m)
                    elif i == 2:
                        # mc1 row 0 = block-2 row 0 (x = 256)
                        nc.vector.tensor_tensor(
                            mirs[1][0:1, 0:HALF], t1s[0:1, :], t2s[0:1, :],
                            op=AOT.subtract,
                        )
                        nc.vector.tensor_tensor(
                            mirs[1][0:1, :][:, N - 1 : HALF - 1 : -1],
                            t1s[0:1, 1 : REV + 1], t2s[0:1, 1 : REV + 1], op=AOT.add,
                        )
                    nc.sync.dma_start(
                        out=out4[:][b][128 * i : 128 * (i + 1), :], in_=ob[:]
                    )
                    if i == 1:
                        # mc0 row 0 = block-1 row 0 (x = 128)
                        nc.scalar.copy(mirs[0][0:1, :], m[0:1, :])
                pending = (b, mirs)
            pb, pmirs = pending
            emit_d(pb, 0, pmirs[1])
            emit_d(pb, 1, pmirs[0])

    nc.compile()
    return nc


def _get_nc():
    global _NC
    if _NC is None:
        _NC = _build()
    return _NC


def _pack_pm(a):
    """[768, W] row-major -> [128, 6*W] with 128-row block j at cols j*W."""
    w = a.shape[1]
    return np.ascontiguousarray(
        a.reshape(NBLK, 128, w).transpose(1, 0, 2).reshape(128, NBLK * w)
    )


def _host_tables():
    a = np.arange(N, dtype=np.int64)
    ang = (2.0 * np.pi / N) * ((a[:, None] * a[None, :]) % N)
    ctv = (np.cos(ang) / N).astype(np.float16)[:, 0:TCOL]
    stv = (np.sin(ang) / N).astype(np.float16)[:, 0:TCOL]
    return _pack_pm(ctv), _pack_pm(stv)


def _host_entry_tables(list_indices, coeff):
    """Bucket each expert's (u, v, coeff) entries by (v-chunk, u-half), pad
    buckets to PAD, lay out partition-major, and interleave u/vm/cv per
    partition so one indirect DMA fetches all three."""
    li = list_indices.astype(np.int64)
    uu = li // N
    vv = li % N
    u2 = np.full((E, EROW), -9.0, np.float32)
    vm2 = np.full((E, EROW), -9.0, np.float32)
    cv2 = np.zeros((E, EROW), np.float32)
    for e in range(E):
        for j in range(NBLK):
            selj = vv[e] // 128 == j
            for ub in range(NBLK):
                sel = np.where(selj & (uu[e] // 128 == ub))[0]
                cnt = len(sel)
                assert cnt <= BPAD, f"bucket overflow: e{e} j{j} ub{ub}: {cnt}"
                base = BROW * j + BPAD * ub
                u2[e, base : base + cnt] = uu[e, sel] - 128 * ub
                vm2[e, base : base + cnt] = vv[e, sel] - 128 * j
                cv2[e, base : base + cnt] = coeff[e, sel]
    # fused layout: ftab[e, p*GCOLS + t*COLS + g] = arr_t[e, 128*g + p]
    ftab = np.zeros((E, 3 * EROW), np.float32)
    p_ix = np.arange(128)[:, None]
    g_ix = np.arange(COLS)[None, :]
    src = (128 * g_ix + p_ix).reshape(-1)          # [128*COLS] entry index
    for t, arr in enumerate((u2, vm2, cv2)):
        dst = (p_ix * GCOLS + t * COLS + g_ix).reshape(-1)
        ftab[:, dst] = arr[:, src]
    return ftab


def kernel(cls_token, W_router, b_router, coeff, list_indices):
    global LAST_RESULT
    cls_token = np.asarray(cls_token)
    W_router = np.asarray(W_router)
    b_router = np.asarray(b_router)
    coeff = np.asarray(coeff)
    list_indices = np.asarray(list_indices)
    assert cls_token.shape == (B, N) and coeff.shape == (E, NF)
    nc = _get_nc()
    ctv, stv = _host_tables()
    ftv = _host_entry_tables(list_indices, coeff)
    basesv = (np.arange(E, dtype=np.float32) * (3 * EROW)).reshape(E, 1)
    jmv = np.zeros((128, 128), np.float16)
    for m_ in range(128):
        jmv[(128 - m_) % 128, m_] = 1.0
    wrtv = _pack_pm(W_router.T.astype(np.float32))
    brr = np.ascontiguousarray(b_router, dtype=np.float32)
    in_maps = []
    for c in range(NCORES):
        in_maps.append(
            {
                "cls4t": _pack_pm(
                    cls_token[BPC * c : BPC * (c + 1)].T.astype(np.float32)
                ),
                "wrt": wrtv,
                "br": brr,
                "ft": ftv,
                "bases": basesv,
                "jm": jmv,
                "ct": ctv,
                "st": stv,
            }
        )
    res = run_bass_kernel_spmd(
        nc, in_maps, core_ids=list(range(NCORES)), trace=KERNEL_TRACE
    )
    LAST_RESULT = res
    out = np.concatenate([res.results[c]["out4"] for c in range(NCORES)], axis=0)
    return out
